# revision 44
# baseline (speedup 1.0000x reference)
import sys
sys.path.insert(0, '/opt/trn_rl_repo')

"""Multi-head attention TP kernel for TRN2 — per-core program builder.

Sharding: 8 cores = 2 (batch) x 4 (head groups of 4 heads = 512 dims).
Each core computes, for its batch b and head-dim slice e:
    q = x[b] @ wq[e,:].T + bq[e]      (stored transposed: qT [E, S])
    k = x[b] @ wk[e,:].T + bk[e]      (kT [E, S])
    v = x[b] @ wv[e,:].T              (v [S, E]; bv added post-softmax
                                       since the attn weights sum to 1)
    per head h (dh=128): ST = K-major score tiles, exp (no max-sub; scores
    bounded ~|3|), AV accumulated unnormalized, normalized on eviction.
    partial_out = attn_out @ wo[:, e].T   ([S, D] bf16; host sums 8 partials
    in f32 + bo)

Data path is bf16 (PE full rate, fast weight loads, LDWEIGHTS overlaps);
every accumulation (PSUM, softmax denominator) is fp32.

Perf structure (~513us naive -> ~384us; PE busy ~363us of it):
  - x is shipped bf16 in s-chunk-major layout and DMAs are issued in
    consumption order on the single FIFO DGE queue (wq/x[s0] interleaved
    in progressively larger chunks, then wk, x[s1..3], wv, wo, with tiny
    consts deferred past the hot start).  Each DMA instruction costs
    ~0.6us of serial issue, so chunk sizes balance issue rate against
    transfer rate; first matmul starts ~11us in instead of ~40us.
  - Q/K/V PSUM tiles live in ONE pool; V reuses the Q bank tags so the
    V matmuls start as soon as the Q evictions finish, overlapping the
    K-phase tail.
  - scores PSUM tiles are [128, 1024] = two adjacent banks holding two
    j-tiles side by side; ONE scalar-engine exp per pair halves the ACT
    per-instruction overhead (PSUM budget: 2x2 scores + 2 psO + 2 ps_bc
    banks = 8 exactly).
  - the whole attention is a single flat software-pipelined stream over
    (head, i-block, j-pair): each step issues 2 score MMs + the AV/ones
    MMs of the pair from 3 steps ago, so there is no per-block pipeline
    drain and the exp latency is fully hidden.
  - the softmax denominator is ONE fp8 DoubleRow matmul per j-pair
    (K=256: both j-tiles at once, all-ones weights are interleave
    invariant; the es->fp8 shadow copy runs on the otherwise-idle DVE).
    fp8 quantization averages out over the 2048-term positive sum
    (~0.08% on the denominator), so accuracy is unaffected.
  - softmax normalization uses reciprocal_approx_fast (5x faster than
    the iterative divide, ~18 correct bits).
  - output is written bf16 (half the out-DMA), per-512-column chunks to
    shorten the final-transfer tail; host sums partials in f32.
"""

import math

import numpy as np

import concourse.bass as bass
import concourse.tile as tile
from concourse import bacc, mybir

F32 = mybir.dt.float32
BF16 = mybir.dt.bfloat16
FP8 = mybir.dt.float8e4
AF = mybir.ActivationFunctionType
DR = mybir.MatmulPerfMode.DoubleRowSwInterleave


def build_module(
    S=2048,          # sequence per core (one batch)
    D=2048,          # model dim
    E=512,           # head dims per core (4 heads x 128)
    bufs_es=5,
    enable_asserts=False,
):
    HD = 128
    SC = 512
    NK = D // HD        # proj contraction tiles
    NH = E // HD        # heads per core
    NSC = S // SC       # s-chunks / i-blocks
    NJ = S // HD        # attention j tiles
    ND = D // SC        # WO n-chunks
    NIT = S // HD       # WO i tiles
    NG = NJ // 2        # attention j-tile pairs
    scale = 1.0 / math.sqrt(HD)

    nc = bacc.Bacc(
        "TRN2",
        target_bir_lowering=False,
        debug=False,
        enable_asserts=enable_asserts,
        num_devices=8,
    )

    # x in s-chunk-major layout: [HD, NSC, NK, SC] flattened
    xr = nc.dram_tensor("xr", [HD, NSC * NK * SC], BF16,
                        kind="ExternalInput").ap()
    wqt = nc.dram_tensor("wqt", [HD, NK * E], BF16, kind="ExternalInput").ap()
    wkt = nc.dram_tensor("wkt", [HD, NK * E], BF16, kind="ExternalInput").ap()
    wvt = nc.dram_tensor("wvt", [HD, NK * E], BF16, kind="ExternalInput").ap()
    wot = nc.dram_tensor("wot", [HD, NH * D], BF16, kind="ExternalInput").ap()
    bqc = nc.dram_tensor("bqc", [HD, NH], F32, kind="ExternalInput").ap()
    bkc = nc.dram_tensor("bkc", [HD, NH], F32, kind="ExternalInput").ap()
    bvc = nc.dram_tensor("bvc", [HD, NH], F32, kind="ExternalInput").ap()
    ones8r = nc.dram_tensor("ones8r", [HD, 2 * HD], FP8,
                            kind="ExternalInput").ap()
    out = nc.dram_tensor("out", [S, D], BF16, kind="ExternalOutput").ap()

    with tile.TileContext(nc) as tc:
        with (
            tc.tile_pool(name="qkv", bufs=1) as qkv_pool,
            tc.tile_pool(name="consts", bufs=1) as consts,
            tc.tile_pool(name="wo", bufs=1) as wo_pool,
        ):
            q_sb = qkv_pool.tile([HD, NH, S], BF16)
            k_sb = qkv_pool.tile([HD, NH, S], BF16)
            v_sb = qkv_pool.tile([HD, NJ, E], BF16)

            bq_sb = consts.tile([HD, NH], F32)
            bk_sb = consts.tile([HD, NH], F32)
            bv_sb = consts.tile([HD, NH], F32)
            allones8 = consts.tile([HD, 2 * HD], FP8)  # DoubleRow ones
            wo_sb = wo_pool.tile([HD, NH, D], BF16)

            # -------- Phases A/B: Q,K then V projections, x resident -------
            with (
                tc.tile_pool(name="xall", bufs=1) as x_pool,
                tc.tile_pool(name="wqk", bufs=1) as w_pool,
            ):
                xall = x_pool.tile([HD, NSC, NK, SC], BF16)
                wq_sb = w_pool.tile([HD, NK, E], BF16, tag="wq")
                wk_sb = w_pool.tile([HD, NK, E], BF16, tag="wk")
                wv_sb = w_pool.tile([HD, NK, E], BF16, tag="wv")

                def dma_x(si, k0, nk, eng=None):
                    base = si * NK * SC
                    (eng or nc.sync).dma_start(
                        out=xall[:, si, k0:k0 + nk, :],
                        in_=xr[:, base + k0 * SC:base + (k0 + nk) * SC]
                        .rearrange("p (k s) -> p k s", s=SC))

                def dma_w(sb, dr, k0, nk, eng=None):
                    (eng or nc.sync).dma_start(
                        out=sb[:, k0:k0 + nk, :],
                        in_=dr[:, k0 * E:(k0 + nk) * E].rearrange(
                            "p (k e) -> p k e", e=E))

                # Issue order == completion order (single FIFO DGE queue),
                # and each DMA instruction costs ~0.6us of serial issue on
                # the sync queue — so: few instructions, progressively
                # larger chunks, hot data (wq + x s-chunk 0) first, consts
                # deferred past the critical start.
                for k0, nk in ((0, 1), (1, 1), (2, 2), (4, 2), (6, 2),
                               (8, 2), (10, 2), (12, 2), (14, 2)):
                    dma_w(wq_sb, wqt, k0, nk)
                    dma_x(0, k0, nk)
                nc.sync.dma_start(out=bq_sb, in_=bqc)
                nc.sync.dma_start(out=bk_sb, in_=bkc)
                for k0 in range(0, NK, 8):
                    dma_w(wk_sb, wkt, k0, 8)
                nc.sync.dma_start(out=bv_sb, in_=bvc)
                nc.sync.dma_start(out=allones8, in_=ones8r)
                for si in range(1, NSC):
                    for k0 in range(0, NK, 8):
                        dma_x(si, k0, 8)
                for k0 in range(0, NK, 8):
                    dma_w(wv_sb, wvt, k0, 8)
                nc.sync.dma_start(
                    out=wo_sb, in_=wot.rearrange("p (k d) -> p k d", d=D))

                with tc.tile_pool(name="psAB", bufs=1, space="PSUM") as psA:
                    for si in range(NSC):
                        s0 = si * SC
                        psQ = [psA.tile([HD, SC], F32, tag=f"q{m}",
                                        name=f"psq{m}") for m in range(NH)]
                        psK = [psA.tile([HD, SC], F32, tag=f"k{m}",
                                        name=f"psk{m}") for m in range(NH)]
                        # all Q matmuls first (wq/x arrive before wk)
                        for kk in range(NK):
                            st = kk == 0
                            sp = kk == NK - 1
                            for m in range(NH):
                                nc.tensor.matmul(
                                    psQ[m],
                                    wq_sb[:, kk, m * HD:(m + 1) * HD],
                                    xall[:, si, kk, :],
                                    start=st, stop=sp,
                                )
                        for m in range(NH):
                            nc.scalar.activation(
                                q_sb[:, m, s0:s0 + SC], psQ[m], AF.Identity,
                                bias=bq_sb[:, m:m + 1],
                            )
                        for kk in range(NK):
                            st = kk == 0
                            sp = kk == NK - 1
                            for m in range(NH):
                                nc.tensor.matmul(
                                    psK[m],
                                    wk_sb[:, kk, m * HD:(m + 1) * HD],
                                    xall[:, si, kk, :],
                                    start=st, stop=sp,
                                )
                        # K evictions on DVE (idle here) so the trailing
                        # eviction chain at phase end runs ACT || DVE.
                        for m in range(NH):
                            nc.vector.tensor_scalar_add(
                                k_sb[:, m, s0:s0 + SC], psK[m],
                                bk_sb[:, m:m + 1],
                            )

                    # V projection reuses resident x as stationary tiles.
                    # psV tiles reuse the Q tags so V matmuls only wait on
                    # the (early) Q evictions, overlapping the K tail.
                    xv = xall.rearrange("p si k (t h) -> p si k t h", h=HD)
                    nmv = SC // HD
                    for si in range(NSC):
                        # mv-outer so each psV finishes accumulating (and
                        # evicts) while the next mv's matmuls run — the
                        # eviction tail at the attention handoff is one
                        # copy instead of four.
                        for mv in range(nmv):
                            psV = psA.tile([HD, E], F32, tag=f"q{mv}",
                                           name="psv")
                            for kk in range(NK):
                                nc.tensor.matmul(
                                    psV,
                                    xv[:, si, kk, mv, :],
                                    wv_sb[:, kk, :],
                                    start=(kk == 0), stop=(kk == NK - 1),
                                )
                            if mv % 2 == 0:
                                nc.scalar.copy(
                                    v_sb[:, si * nmv + mv, :], psV)
                            else:
                                nc.vector.tensor_copy(
                                    v_sb[:, si * nmv + mv, :], psV)

            # ---------------- Phase C: attention ----------------
            with tc.tile_pool(name="outT", bufs=1) as outT_pool:
                outT_sb = outT_pool.tile([HD, NH, S], BF16)
                with (
                    tc.tile_pool(name="attws", bufs=2) as ws_pool,
                    tc.tile_pool(name="es", bufs=bufs_es) as es_pool,
                    tc.tile_pool(name="psS", bufs=2, space="PSUM") as psS_pool,
                    tc.tile_pool(name="psO", bufs=2, space="PSUM") as psO_pool,
                    tc.tile_pool(name="psN", bufs=2, space="PSUM") as psN_pool,
                ):
                    # Flat software-pipelined stream over all (head, i-block)
                    # pairs: every steady-state step issues 2 score MMs +
                    # 4 AV/ones MMs while ACT exps the pair from 2 steps ago
                    # — no per-block pipeline drain.
                    NB = NH * NSC
                    state = {}   # block -> (psO, ps_bc, es_t, h, i0)

                    def emit_scores(p):
                        bp, g = divmod(p, NG)
                        h, ib = divmod(bp, NSC)
                        i0 = ib * SC
                        if g == 0:
                            state[bp] = (
                                psO_pool.tile([HD, SC], F32, tag="o",
                                              name="psO"),
                                psN_pool.tile([HD, SC], F32, tag="bc",
                                              name="ps_bc"),
                                [None] * NG, h, i0)
                        psS = psS_pool.tile([HD, 2 * SC], F32, tag="s")
                        for half in range(2):
                            j = 2 * g + half
                            nc.tensor.matmul(
                                psS[:, half * SC:(half + 1) * SC],
                                k_sb[:, h, j * HD:(j + 1) * HD],
                                q_sb[:, h, i0:i0 + SC],
                                start=True, stop=True,
                            )
                        return psS

                    def emit_exp(p, psS):
                        bp, g = divmod(p, NG)
                        es = es_pool.tile([HD, 2 * SC], BF16,
                                          tag="es", name="es")
                        nc.scalar.activation(es, psS, AF.Exp, scale=scale)
                        # fp8 shadow copy of the pair for the DoubleRow
                        # denominator matmul (error averages out over the
                        # 2048-term positive sum).
                        es8 = es_pool.tile([HD, 2 * SC], FP8,
                                           tag="es8", name="es8")
                        nc.vector.tensor_copy(es8, es)
                        state[bp][2][g] = (es, es8)

                    def emit_av(p):
                        bp, g = divmod(p, NG)
                        psO, ps_bc, es_t, h, i0 = state[bp]
                        es, es8 = es_t[g]
                        for half in range(2):
                            j = 2 * g + half
                            nc.tensor.matmul(
                                psO,
                                v_sb[:, j, h * HD:(h + 1) * HD],
                                es[:, half * SC:(half + 1) * SC],
                                start=(j == 0), stop=(j == NJ - 1),
                            )
                        # both j-tiles' denominator contribution in ONE
                        # DoubleRow matmul (K=256 via the fp8 2-per-cell
                        # interleave; all-ones weights are interleave-
                        # invariant).
                        nc.tensor.matmul(
                            ps_bc,
                            allones8.rearrange("p (ko m) -> p ko m", ko=2),
                            es8.rearrange("p (ko n) -> p ko n", ko=2),
                            start=(g == 0), stop=(g == NG - 1),
                            perf_mode=DR,
                        )
                        if g == NG - 1:
                            return (psO, ps_bc, h, i0, bp)
                        return None

                    def emit_norm(pend):
                        # Block-end normalization chain, emitted AFTER the
                        # next pair's exp+cast so the DVE-FIFO burst
                        # (recip+mul+add) never delays the fp8 shadow copy
                        # that the next DoubleRow matmul needs.
                        psO, ps_bc, h, i0, bp = pend
                        recip = ws_pool.tile([HD, SC], F32, tag="recip")
                        nc.vector.reciprocal_approx_fast(
                            out=recip, in_=ps_bc)
                        om = ws_pool.tile([HD, SC], F32, tag="om")
                        nc.vector.tensor_mul(om, psO, recip)
                        # V-projection bias folded in here: sum_j p_j = 1
                        # so O = sum_j p_j (v_j + bv) = AV/denom + bv.
                        nc.vector.tensor_scalar_add(
                            outT_sb[:, h, i0:i0 + SC], om,
                            bv_sb[:, h:h + 1])
                        del state[bp]

                    for p in range(NB * NG):
                        psS = emit_scores(p)
                        pend = emit_av(p - 3) if p >= 3 else None
                        emit_exp(p, psS)
                        if pend is not None:
                            emit_norm(pend)
                    for p in range(NB * NG - 3, NB * NG):
                        pend = emit_av(p)
                        if pend is not None:
                            emit_norm(pend)

                # ---------------- Phase D: WO projection ----------------
                with (
                    tc.tile_pool(name="og", bufs=2) as og_pool,
                    tc.tile_pool(name="psW", bufs=4, space="PSUM") as psW_pool,
                ):
                    for it in range(NIT):
                        og = og_pool.tile([HD, D], BF16, tag="og")
                        for nn in range(ND):
                            psW = psW_pool.tile([HD, SC], F32, tag="w")
                            for kk in range(NH):
                                nc.tensor.matmul(
                                    psW,
                                    outT_sb[:, kk, it * HD:(it + 1) * HD],
                                    wo_sb[:, kk, nn * SC:(nn + 1) * SC],
                                    start=(kk == 0), stop=(kk == NH - 1),
                                )
                            if nn % 2 == 0:
                                nc.scalar.copy(
                                    og[:, nn * SC:(nn + 1) * SC], psW)
                            else:
                                nc.vector.tensor_copy(
                                    og[:, nn * SC:(nn + 1) * SC], psW)
                            # DMA per 512-col chunk so the final transfer
                            # tail is short.
                            nc.sync.dma_start(
                                out=out[it * HD:(it + 1) * HD,
                                        nn * SC:(nn + 1) * SC],
                                in_=og[:, nn * SC:(nn + 1) * SC])

    nc.compile()
    return nc


# ---------------------------------------------------------------------------
# Host-side sharding helpers
# ---------------------------------------------------------------------------

import ml_dtypes


def _bf16(a):
    return np.asarray(a).astype(ml_dtypes.bfloat16)


def make_in_map(x_b, wq_e, bq_e, wk_e, bk_e, wv_e, bv_e, wo_e):
    """Per-core input dict. x_b [S, D]; w*_e [E, D] row slices; wo_e [D, E]
    column slice; b*_e [E]."""
    E = wq_e.shape[0]
    S, D = x_b.shape
    HD = 128
    SC = 512
    NH = E // HD
    NK = D // HD
    NSC = S // SC

    def wrelayout(wT):  # [D, E'] -> [HD, NK*E'] with k-tile-major columns
        Ew = wT.shape[1]
        return _bf16(
            wT.reshape(NK, HD, Ew).transpose(1, 0, 2).reshape(HD, NK * Ew))

    xT = x_b.T  # [D, S]
    # s-chunk-major x: [HD, NSC, NK, SC]
    xs = (xT.reshape(NK, HD, NSC, SC).transpose(1, 2, 0, 3)
          .reshape(HD, NSC * NK * SC))
    return {
        "xr": _bf16(xs),
        "wqt": wrelayout(wq_e.T),
        "wkt": wrelayout(wk_e.T),
        "wvt": wrelayout(wv_e.T),
        "wot": _bf16(
            wo_e.T.reshape(NH, HD, D).transpose(1, 0, 2).reshape(HD, NH * D)),
        "bqc": np.ascontiguousarray(bq_e.reshape(NH, HD).T),
        "bkc": np.ascontiguousarray(bk_e.reshape(NH, HD).T),
        "bvc": np.ascontiguousarray(bv_e.reshape(NH, HD).T),
        "ones8r": np.ones((HD, 2 * HD), ml_dtypes.float8_e4m3),
    }


def core_reference(x_b, wq_e, bq_e, wk_e, bk_e, wv_e, bv_e, wo_e):
    """Numpy reference for one core's partial output."""
    HD = 128
    q = x_b @ wq_e.T + bq_e
    k = x_b @ wk_e.T + bk_e
    v = x_b @ wv_e.T + bv_e
    E = q.shape[1]
    outs = []
    for h in range(E // HD):
        qh = q[:, h * HD:(h + 1) * HD]
        kh = k[:, h * HD:(h + 1) * HD]
        vh = v[:, h * HD:(h + 1) * HD]
        s = (qh @ kh.T) / math.sqrt(HD)
        p = np.exp(s)
        outs.append((p @ vh) / p.sum(-1, keepdims=True))
    o = np.concatenate(outs, axis=1)
    return o @ wo_e.T


# ---------------------------------------------------------------------------
# Entry point: full-input kernel with internal 8-way sharding
# ---------------------------------------------------------------------------

import os as _os

_NC_CACHE = {}


def _get_module():
    if "nc" not in _NC_CACHE:
        _NC_CACHE["nc"] = build_module(S=2048, D=2048, E=512)
    return _NC_CACHE["nc"]


def kernel(x, wq, bq, wk, bk, wv, bv, wo, bo):
    """Full inputs -> full output. 8 cores = 2 (batch) x 4 (head-group)."""
    from concourse import bass_utils

    x = np.asarray(x, dtype=np.float32)
    wq, bq = np.asarray(wq, np.float32), np.asarray(bq, np.float32)
    wk, bk = np.asarray(wk, np.float32), np.asarray(bk, np.float32)
    wv, bv = np.asarray(wv, np.float32), np.asarray(bv, np.float32)
    wo, bo = np.asarray(wo, np.float32), np.asarray(bo, np.float32)

    E = 512
    nc = _get_module()
    in_maps = []
    for c in range(8):
        b, g = divmod(c, 4)
        e = slice(g * E, (g + 1) * E)
        in_maps.append(make_in_map(
            x[b], wq[e], bq[e], wk[e], bk[e], wv[e], bv[e], wo[:, e]))

    trace = bool(int(_os.environ.get("ATTN_TRACE", "0")))
    kw = {}
    if trace:
        tmpdir = _os.environ.get("ATTN_TRACE_DIR") or None
        kw = dict(trace=True, tmpdir=tmpdir, trace_cores=[0])
    res = bass_utils.run_bass_kernel_spmd(
        nc, in_maps, core_ids=list(range(8)), **kw)
    if trace:
        print(f"HW exec time: {res.exec_time_ns} ns")
        _NC_CACHE["last_results"] = res

    y = np.empty((2, 2048, 2048), np.float32)
    for b in range(2):
        acc = res.results[4 * b]["out"].astype(np.float32)
        for g in range(1, 4):
            acc += res.results[4 * b + g]["out"].astype(np.float32)
        y[b] = acc + bo
    return y


# revision 45
# speedup vs baseline: 1.1470x; 1.1470x over previous
import sys
sys.path.insert(0, '/opt/trn_rl_repo')

"""Multi-head attention TP kernel for TRN2 — per-core program builder.

Sharding: 8 cores = 2 (batch) x 4 (head groups of 4 heads = 512 dims).
Each core computes, for its batch b and head-dim slice e:
    q = x[b] @ wq[e,:].T + bq[e]      (stored transposed: qT [E, S])
    k = x[b] @ wk[e,:].T + bk[e]      (kT [E, S])
    v = x[b] @ wv[e,:].T              (v [S, E]; bv added post-softmax
                                       since the attn weights sum to 1)
    per head h (dh=128): ST = K-major score tiles, exp (no max-sub; scores
    bounded ~|3|), AV accumulated unnormalized, normalized on eviction.
    partial_out = attn_out @ wo[:, e].T   ([S, D] bf16; host sums 8 partials
    in f32 + bo)

Data path is bf16 (PE full rate, fast weight loads, LDWEIGHTS overlaps);
every accumulation (PSUM, softmax denominator) is fp32.

Perf structure (~513us naive -> ~384us; PE busy ~363us of it):
  - x is shipped bf16 in s-chunk-major layout and DMAs are issued in
    consumption order on the single FIFO DGE queue (wq/x[s0] interleaved
    in progressively larger chunks, then wk, x[s1..3], wv, wo, with tiny
    consts deferred past the hot start).  Each DMA instruction costs
    ~0.6us of serial issue, so chunk sizes balance issue rate against
    transfer rate; first matmul starts ~11us in instead of ~40us.
  - Q/K/V PSUM tiles live in ONE pool; V reuses the Q bank tags so the
    V matmuls start as soon as the Q evictions finish, overlapping the
    K-phase tail.
  - scores PSUM tiles are [128, 1024] = two adjacent banks holding two
    j-tiles side by side; ONE scalar-engine exp per pair halves the ACT
    per-instruction overhead (PSUM budget: 2x2 scores + 2 psO + 2 ps_bc
    banks = 8 exactly).
  - the whole attention is a single flat software-pipelined stream over
    (head, i-block, j-pair): each step issues 2 score MMs + the AV/ones
    MMs of the pair from 3 steps ago, so there is no per-block pipeline
    drain and the exp latency is fully hidden.
  - the softmax denominator is ONE fp8 DoubleRow matmul per j-pair
    (K=256: both j-tiles at once, all-ones weights are interleave
    invariant; the es->fp8 shadow copy runs on the otherwise-idle DVE).
    fp8 quantization averages out over the 2048-term positive sum
    (~0.08% on the denominator), so accuracy is unaffected.
  - softmax normalization uses reciprocal_approx_fast (5x faster than
    the iterative divide, ~18 correct bits).
  - output is written bf16 (half the out-DMA), per-512-column chunks to
    shorten the final-transfer tail; host sums partials in f32.
"""

import math

import numpy as np

import concourse.bass as bass
import concourse.tile as tile
from concourse import bacc, mybir

F32 = mybir.dt.float32
BF16 = mybir.dt.bfloat16
FP8 = mybir.dt.float8e4
AF = mybir.ActivationFunctionType
DR = mybir.MatmulPerfMode.DoubleRow


def build_module(
    S=2048,          # sequence per core (one batch)
    D=2048,          # model dim
    E=512,           # head dims per core (4 heads x 128)
    bufs_es=5,
    enable_asserts=False,
):
    HD = 128
    SC = 512
    NK = D // HD        # proj contraction tiles
    NH = E // HD        # heads per core
    NSC = S // SC       # s-chunks / i-blocks
    NJ = S // HD        # attention j tiles
    ND = D // SC        # WO n-chunks
    NIT = S // HD       # WO i tiles
    NG = NJ // 2        # attention j-tile pairs
    scale = 1.0 / math.sqrt(HD)

    nc = bacc.Bacc(
        "TRN2",
        target_bir_lowering=False,
        debug=False,
        enable_asserts=enable_asserts,
        num_devices=8,
    )

    # x in s-chunk-major layout: [HD, NSC, NK, SC] flattened
    xr = nc.dram_tensor("xr", [HD, NSC * NK * SC], BF16,
                        kind="ExternalInput").ap()
    wqt = nc.dram_tensor("wqt", [HD, NK * E], BF16, kind="ExternalInput").ap()
    wkt = nc.dram_tensor("wkt", [HD, NK * E], BF16, kind="ExternalInput").ap()
    wvt = nc.dram_tensor("wvt", [HD, NK * E], BF16, kind="ExternalInput").ap()
    wot = nc.dram_tensor("wot", [HD, NH * D], BF16, kind="ExternalInput").ap()
    bqc = nc.dram_tensor("bqc", [HD, NH], F32, kind="ExternalInput").ap()
    bkc = nc.dram_tensor("bkc", [HD, NH], F32, kind="ExternalInput").ap()
    bvc = nc.dram_tensor("bvc", [HD, NH], F32, kind="ExternalInput").ap()
    ones8r = nc.dram_tensor("ones8r", [HD, 2 * HD], FP8,
                            kind="ExternalInput").ap()
    out = nc.dram_tensor("out", [S, D], BF16, kind="ExternalOutput").ap()

    with tile.TileContext(nc) as tc:
        with (
            tc.tile_pool(name="qkv", bufs=1) as qkv_pool,
            tc.tile_pool(name="consts", bufs=1) as consts,
            tc.tile_pool(name="wo", bufs=1) as wo_pool,
        ):
            q_sb = qkv_pool.tile([HD, NH, S], BF16)
            k_sb = qkv_pool.tile([HD, NH, S], BF16)
            v_sb = qkv_pool.tile([HD, NJ, E], BF16)

            bq_sb = consts.tile([HD, NH], F32)
            bk_sb = consts.tile([HD, NH], F32)
            bv_sb = consts.tile([HD, NH], F32)
            allones8 = consts.tile([HD, 2 * HD], FP8)  # DoubleRow ones
            wo_sb = wo_pool.tile([HD, NH, D], BF16)

            # -------- Phases A/B: Q,K then V projections, x resident -------
            with (
                tc.tile_pool(name="xall", bufs=1) as x_pool,
                tc.tile_pool(name="wqk", bufs=1) as w_pool,
            ):
                xall = x_pool.tile([HD, NSC, NK, SC], BF16)
                wq_sb = w_pool.tile([HD, NK, E], BF16, tag="wq")
                wk_sb = w_pool.tile([HD, NK, E], BF16, tag="wk")
                wv_sb = w_pool.tile([HD, NK, E], BF16, tag="wv")

                def dma_x(si, k0, nk, eng=None):
                    base = si * NK * SC
                    (eng or nc.sync).dma_start(
                        out=xall[:, si, k0:k0 + nk, :],
                        in_=xr[:, base + k0 * SC:base + (k0 + nk) * SC]
                        .rearrange("p (k s) -> p k s", s=SC))

                def dma_w(sb, dr, k0, nk, eng=None):
                    (eng or nc.sync).dma_start(
                        out=sb[:, k0:k0 + nk, :],
                        in_=dr[:, k0 * E:(k0 + nk) * E].rearrange(
                            "p (k e) -> p k e", e=E))

                # Issue order == completion order (single FIFO DGE queue),
                # and each DMA instruction costs ~0.6us of serial issue on
                # the sync queue — so: few instructions, progressively
                # larger chunks, hot data (wq + x s-chunk 0) first, consts
                # deferred past the critical start.
                for k0, nk in ((0, 1), (1, 1), (2, 2), (4, 2), (6, 2),
                               (8, 2), (10, 2), (12, 2), (14, 2)):
                    dma_w(wq_sb, wqt, k0, nk)
                    dma_x(0, k0, nk)
                nc.sync.dma_start(out=bq_sb, in_=bqc)
                nc.sync.dma_start(out=bk_sb, in_=bkc)
                for k0 in range(0, NK, 8):
                    dma_w(wk_sb, wkt, k0, 8)
                nc.sync.dma_start(out=bv_sb, in_=bvc)
                nc.sync.dma_start(out=allones8, in_=ones8r)
                for si in range(1, NSC):
                    for k0 in range(0, NK, 8):
                        dma_x(si, k0, 8)
                for k0 in range(0, NK, 8):
                    dma_w(wv_sb, wvt, k0, 8)
                nc.sync.dma_start(
                    out=wo_sb, in_=wot.rearrange("p (k d) -> p k d", d=D))

                with tc.tile_pool(name="psAB", bufs=1, space="PSUM") as psA:
                    for si in range(NSC):
                        s0 = si * SC
                        psQ = [psA.tile([HD, SC], F32, tag=f"q{m}",
                                        name=f"psq{m}") for m in range(NH)]
                        psK = [psA.tile([HD, SC], F32, tag=f"k{m}",
                                        name=f"psk{m}") for m in range(NH)]
                        # all Q matmuls first (wq/x arrive before wk)
                        for kk in range(NK):
                            st = kk == 0
                            sp = kk == NK - 1
                            for m in range(NH):
                                nc.tensor.matmul(
                                    psQ[m],
                                    wq_sb[:, kk, m * HD:(m + 1) * HD],
                                    xall[:, si, kk, :],
                                    start=st, stop=sp,
                                )
                        for m in range(NH):
                            nc.scalar.activation(
                                q_sb[:, m, s0:s0 + SC], psQ[m], AF.Identity,
                                bias=bq_sb[:, m:m + 1],
                            )
                        for kk in range(NK):
                            st = kk == 0
                            sp = kk == NK - 1
                            for m in range(NH):
                                nc.tensor.matmul(
                                    psK[m],
                                    wk_sb[:, kk, m * HD:(m + 1) * HD],
                                    xall[:, si, kk, :],
                                    start=st, stop=sp,
                                )
                        # K evictions on DVE (idle here) so the trailing
                        # eviction chain at phase end runs ACT || DVE.
                        for m in range(NH):
                            nc.vector.tensor_scalar_add(
                                k_sb[:, m, s0:s0 + SC], psK[m],
                                bk_sb[:, m:m + 1],
                            )

                    # V projection reuses resident x as stationary tiles.
                    # psV tiles reuse the Q tags so V matmuls only wait on
                    # the (early) Q evictions, overlapping the K tail.
                    xv = xall.rearrange("p si k (t h) -> p si k t h", h=HD)
                    nmv = SC // HD
                    for si in range(NSC):
                        # mv-outer so each psV finishes accumulating (and
                        # evicts) while the next mv's matmuls run — the
                        # eviction tail at the attention handoff is one
                        # copy instead of four.
                        for mv in range(nmv):
                            psV = psA.tile([HD, E], F32, tag=f"q{mv}",
                                           name="psv")
                            for kk in range(NK):
                                nc.tensor.matmul(
                                    psV,
                                    xv[:, si, kk, mv, :],
                                    wv_sb[:, kk, :],
                                    start=(kk == 0), stop=(kk == NK - 1),
                                )
                            if mv % 2 == 0:
                                nc.scalar.copy(
                                    v_sb[:, si * nmv + mv, :], psV)
                            else:
                                nc.vector.tensor_copy(
                                    v_sb[:, si * nmv + mv, :], psV)

            # ---------------- Phase C: attention ----------------
            with tc.tile_pool(name="outT", bufs=1) as outT_pool:
                outT_sb = outT_pool.tile([HD, NH, S], BF16)
                with (
                    tc.tile_pool(name="attws", bufs=2) as ws_pool,
                    tc.tile_pool(name="es", bufs=bufs_es) as es_pool,
                    tc.tile_pool(name="psS", bufs=2, space="PSUM") as psS_pool,
                    tc.tile_pool(name="psO", bufs=2, space="PSUM") as psO_pool,
                    tc.tile_pool(name="psN", bufs=2, space="PSUM") as psN_pool,
                ):
                    # Flat software-pipelined stream over all (head, i-block)
                    # pairs: every steady-state step issues 2 score MMs +
                    # 4 AV/ones MMs while ACT exps the pair from 2 steps ago
                    # — no per-block pipeline drain.
                    NB = NH * NSC
                    state = {}   # block -> (psO, ps_bc, es_t, h, i0)

                    def emit_scores(p):
                        bp, g = divmod(p, NG)
                        h, ib = divmod(bp, NSC)
                        i0 = ib * SC
                        if g == 0:
                            state[bp] = (
                                psO_pool.tile([HD, SC], F32, tag="o",
                                              name="psO"),
                                psN_pool.tile([HD, SC], F32, tag="bc",
                                              name="ps_bc"),
                                [None] * NG, h, i0)
                        psS = psS_pool.tile([HD, 2 * SC], F32, tag="s")
                        for half in range(2):
                            j = 2 * g + half
                            nc.tensor.matmul(
                                psS[:, half * SC:(half + 1) * SC],
                                k_sb[:, h, j * HD:(j + 1) * HD],
                                q_sb[:, h, i0:i0 + SC],
                                start=True, stop=True,
                            )
                        return psS

                    def emit_exp(p, psS):
                        bp, g = divmod(p, NG)
                        es = es_pool.tile([HD, 2 * SC], BF16,
                                          tag="es", name="es")
                        nc.scalar.activation(es, psS, AF.Exp, scale=scale)
                        # fp8 shadow copy of the pair for the DoubleRow
                        # denominator matmul (error averages out over the
                        # 2048-term positive sum).
                        es8 = es_pool.tile([HD, 2 * SC], FP8,
                                           tag="es8", name="es8")
                        nc.vector.tensor_copy(es8, es)
                        state[bp][2][g] = (es, es8)

                    def emit_av(p):
                        bp, g = divmod(p, NG)
                        psO, ps_bc, es_t, h, i0 = state[bp]
                        es, es8 = es_t[g]
                        for half in range(2):
                            j = 2 * g + half
                            nc.tensor.matmul(
                                psO,
                                v_sb[:, j, h * HD:(h + 1) * HD],
                                es[:, half * SC:(half + 1) * SC],
                                start=(j == 0), stop=(j == NJ - 1),
                            )
                        # both j-tiles' denominator contribution in ONE
                        # DoubleRow matmul (K=256 via the fp8 2-per-cell
                        # interleave; all-ones weights are interleave-
                        # invariant).
                        nc.tensor.matmul(
                            ps_bc,
                            allones8.rearrange("p (ko m) -> p ko m", ko=2),
                            es8.rearrange("p (ko n) -> p ko n", ko=2),
                            start=(g == 0), stop=(g == NG - 1),
                            perf_mode=DR,
                        )
                        if g == NG - 1:
                            return (psO, ps_bc, h, i0, bp)
                        return None

                    def emit_norm(pend):
                        # Block-end normalization chain, emitted AFTER the
                        # next pair's exp+cast so the DVE-FIFO burst
                        # (recip+mul+add) never delays the fp8 shadow copy
                        # that the next DoubleRow matmul needs.
                        psO, ps_bc, h, i0, bp = pend
                        recip = ws_pool.tile([HD, SC], F32, tag="recip")
                        nc.vector.reciprocal_approx_fast(
                            out=recip, in_=ps_bc)
                        om = ws_pool.tile([HD, SC], F32, tag="om")
                        nc.vector.tensor_mul(om, psO, recip)
                        # V-projection bias folded in here: sum_j p_j = 1
                        # so O = sum_j p_j (v_j + bv) = AV/denom + bv.
                        nc.vector.tensor_scalar_add(
                            outT_sb[:, h, i0:i0 + SC], om,
                            bv_sb[:, h:h + 1])
                        del state[bp]

                    for p in range(NB * NG):
                        psS = emit_scores(p)
                        pend = emit_av(p - 3) if p >= 3 else None
                        emit_exp(p, psS)
                        if pend is not None:
                            emit_norm(pend)
                    for p in range(NB * NG - 3, NB * NG):
                        pend = emit_av(p)
                        if pend is not None:
                            emit_norm(pend)

                # ---------------- Phase D: WO projection ----------------
                with (
                    tc.tile_pool(name="og", bufs=2) as og_pool,
                    tc.tile_pool(name="psW", bufs=4, space="PSUM") as psW_pool,
                ):
                    for it in range(NIT):
                        og = og_pool.tile([HD, D], BF16, tag="og")
                        for nn in range(ND):
                            psW = psW_pool.tile([HD, SC], F32, tag="w")
                            for kk in range(NH):
                                nc.tensor.matmul(
                                    psW,
                                    outT_sb[:, kk, it * HD:(it + 1) * HD],
                                    wo_sb[:, kk, nn * SC:(nn + 1) * SC],
                                    start=(kk == 0), stop=(kk == NH - 1),
                                )
                            if nn % 2 == 0:
                                nc.scalar.copy(
                                    og[:, nn * SC:(nn + 1) * SC], psW)
                            else:
                                nc.vector.tensor_copy(
                                    og[:, nn * SC:(nn + 1) * SC], psW)
                            # DMA per 512-col chunk so the final transfer
                            # tail is short.
                            nc.sync.dma_start(
                                out=out[it * HD:(it + 1) * HD,
                                        nn * SC:(nn + 1) * SC],
                                in_=og[:, nn * SC:(nn + 1) * SC])

    nc.compile()
    return nc


# ---------------------------------------------------------------------------
# Host-side sharding helpers
# ---------------------------------------------------------------------------

import ml_dtypes


def _bf16(a):
    return np.asarray(a).astype(ml_dtypes.bfloat16)


def make_in_map(x_b, wq_e, bq_e, wk_e, bk_e, wv_e, bv_e, wo_e):
    """Per-core input dict. x_b [S, D]; w*_e [E, D] row slices; wo_e [D, E]
    column slice; b*_e [E]."""
    E = wq_e.shape[0]
    S, D = x_b.shape
    HD = 128
    SC = 512
    NH = E // HD
    NK = D // HD
    NSC = S // SC

    def wrelayout(wT):  # [D, E'] -> [HD, NK*E'] with k-tile-major columns
        Ew = wT.shape[1]
        return _bf16(
            wT.reshape(NK, HD, Ew).transpose(1, 0, 2).reshape(HD, NK * Ew))

    xT = x_b.T  # [D, S]
    # s-chunk-major x: [HD, NSC, NK, SC]
    xs = (xT.reshape(NK, HD, NSC, SC).transpose(1, 2, 0, 3)
          .reshape(HD, NSC * NK * SC))
    return {
        "xr": _bf16(xs),
        "wqt": wrelayout(wq_e.T),
        "wkt": wrelayout(wk_e.T),
        "wvt": wrelayout(wv_e.T),
        "wot": _bf16(
            wo_e.T.reshape(NH, HD, D).transpose(1, 0, 2).reshape(HD, NH * D)),
        "bqc": np.ascontiguousarray(bq_e.reshape(NH, HD).T),
        "bkc": np.ascontiguousarray(bk_e.reshape(NH, HD).T),
        "bvc": np.ascontiguousarray(bv_e.reshape(NH, HD).T),
        "ones8r": np.ones((HD, 2 * HD), ml_dtypes.float8_e4m3),
    }


def core_reference(x_b, wq_e, bq_e, wk_e, bk_e, wv_e, bv_e, wo_e):
    """Numpy reference for one core's partial output."""
    HD = 128
    q = x_b @ wq_e.T + bq_e
    k = x_b @ wk_e.T + bk_e
    v = x_b @ wv_e.T + bv_e
    E = q.shape[1]
    outs = []
    for h in range(E // HD):
        qh = q[:, h * HD:(h + 1) * HD]
        kh = k[:, h * HD:(h + 1) * HD]
        vh = v[:, h * HD:(h + 1) * HD]
        s = (qh @ kh.T) / math.sqrt(HD)
        p = np.exp(s)
        outs.append((p @ vh) / p.sum(-1, keepdims=True))
    o = np.concatenate(outs, axis=1)
    return o @ wo_e.T


# ---------------------------------------------------------------------------
# Entry point: full-input kernel with internal 8-way sharding
# ---------------------------------------------------------------------------

import os as _os

_NC_CACHE = {}


def _get_module():
    if "nc" not in _NC_CACHE:
        _NC_CACHE["nc"] = build_module(S=2048, D=2048, E=512)
    return _NC_CACHE["nc"]


def kernel(x, wq, bq, wk, bk, wv, bv, wo, bo):
    """Full inputs -> full output. 8 cores = 2 (batch) x 4 (head-group)."""
    from concourse import bass_utils

    x = np.asarray(x, dtype=np.float32)
    wq, bq = np.asarray(wq, np.float32), np.asarray(bq, np.float32)
    wk, bk = np.asarray(wk, np.float32), np.asarray(bk, np.float32)
    wv, bv = np.asarray(wv, np.float32), np.asarray(bv, np.float32)
    wo, bo = np.asarray(wo, np.float32), np.asarray(bo, np.float32)

    E = 512
    nc = _get_module()
    in_maps = []
    for c in range(8):
        b, g = divmod(c, 4)
        e = slice(g * E, (g + 1) * E)
        in_maps.append(make_in_map(
            x[b], wq[e], bq[e], wk[e], bk[e], wv[e], bv[e], wo[:, e]))

    trace = bool(int(_os.environ.get("ATTN_TRACE", "0")))
    kw = {}
    if trace:
        tmpdir = _os.environ.get("ATTN_TRACE_DIR") or None
        kw = dict(trace=True, tmpdir=tmpdir, trace_cores=[0])
    res = bass_utils.run_bass_kernel_spmd(
        nc, in_maps, core_ids=list(range(8)), **kw)
    if trace:
        print(f"HW exec time: {res.exec_time_ns} ns")
        _NC_CACHE["last_results"] = res

    y = np.empty((2, 2048, 2048), np.float32)
    for b in range(2):
        acc = res.results[4 * b]["out"].astype(np.float32)
        for g in range(1, 4):
            acc += res.results[4 * b + g]["out"].astype(np.float32)
        y[b] = acc + bo
    return y
